# revision 18
# baseline (speedup 1.0000x reference)
"""BlockGrouper (MoE routing dispatch) Trainium2 kernel — raw bass.

Semantics (from the reference): each token n in sample b belongs to group
g = argmax(block_onehot[b, n]); its slot within the group is its rank
among same-group tokens in token order.  With the balanced one-hot
routing, the output [B, G, cap, D] is a pure row-permutation of
x [B, N, D].

Sharding: data-parallel over B across the 8 NeuronCores (one sample per
core); each core moves 16 MiB in + 16 MiB out.

Per-core program (N=8192, G=16, D=512, cap=512, P=128, C=64; token n
lives at partition p = n // 64, column c = n % 64):
  1. Index pipeline: dest[n] = g*cap + rank(n)
     = sum_g onehot * (prefix_c + carry_p + g*cap - 1), where prefix_c is
     a per-partition inclusive prefix sum along c (16 strided
     tensor_tensor_scan ops) and carry_p comes from one
     strict-upper-triangular-ones matmul on the per-partition totals
     (plus a const-row matmul adding g*cap - 1).  dest_f (f32) is cast
     to dest_i (int32) for the indirect scatters, and folded to the
     int16 SWDGE layout (destw) for the two dma_scatter_add chunks.
  2. Data path, two mechanisms that attack different bottlenecks:
     - 48 columns go out via indirect scatter-WRITE DMAs
       (indirect_dma_start, cce bypass): one plain 2 KiB descriptor per
       row, minimal DMA-engine work, but each 128-row call costs
       ~1.4 us of Pool-engine issue time (the critical path).
     - 16 columns (c=0..15) go out via two canned dma_scatter_add ops
       (1024 rows per instruction, ~2 us issue each) on SWDGE queues
       1/2 while the indirect stream keeps queue 0 — trading some CCE
       read-modify-write DMA work (engines have headroom) to shorten
       the serialized issue stream by ~23 us.
     x loads are 1 MiB groups (4 columns, 8 KiB contiguous per
     partition) ordered so the columns the scatter stream consumes
     first arrive first (g4,g6,g0,g2,... on ACT; g5,g7,g1,g3,... on SP
     behind the onehot load).
  A dummy 16-index scatter at t=0 preloads the Q7 extended-instruction
  library (LOAD_LIB costs ~13 us if taken lazily).
"""


import numpy as np

B, N, G, D = 8, 8192, 16, 512
CAP = N // G
P = 128
C = N // P
NCORES = 8
NG = 16          # load groups
GC = C // NG     # 4 columns per load group
GCH = 1024       # rows per dma_scatter_add chunk (8 columns)

_cached = None


def _build():
    import concourse.bass as bass
    import concourse.bacc as bacc
    import concourse.mybir as mybir

    f32 = mybir.dt.float32
    i32 = mybir.dt.int32
    i16 = mybir.dt.int16

    nc = bacc.Bacc("TRN2", target_bir_lowering=False, debug=False,
                   num_devices=NCORES, num_swdge_queues=4)
    x_d = nc.dram_tensor("x", [N, D], f32, kind="ExternalInput")
    oh_d = nc.dram_tensor("oh", [N, G], f32, kind="ExternalInput")
    cst_big_d = nc.dram_tensor("cst_big", [P, 9 * P], f32,
                               kind="ExternalInput")
    cst_row_d = nc.dram_tensor("cst_row", [1, P + G], f32,
                               kind="ExternalInput")
    out_d = nc.dram_tensor("out", [N, D], f32, kind="ExternalOutput")
    dummy_d = nc.dram_tensor("lib_warm", [16, 64], f32,
                             kind="ExternalOutput")

    # load segments (column ranges) per ring, in emission order: the
    # columns the scatter stream consumes first (16..23) land first in
    # small 4-column pieces; the dma_scatter_add chunks (0..15) next;
    # the rest as big 8-column segments.  Few DMA instructions keep the
    # program-init shape-register cost down.
    act_segs = [(16, 20), (0, 8), (24, 28), (32, 40), (48, 56)]
    sp_segs = [(20, 24), (8, 16), (28, 32), (40, 48), (56, 64)]

    from contextlib import ExitStack
    with ExitStack() as stack:
        ec = stack.enter_context
        cst_big_t = ec(nc.sbuf_tensor("cst_big_t", [P, 9 * P], f32))
        cst_row_t = ec(nc.sbuf_tensor("cst_row_t", [1, P + G], f32))
        oh_t = ec(nc.sbuf_tensor("oh_t", [P, C * G], f32))
        scan_t = ec(nc.sbuf_tensor("scan_t", [P, C * G], f32))
        s_t = ec(nc.sbuf_tensor("s_t", [P, C * G], f32))
        prod_t = ec(nc.sbuf_tensor("prod_t", [P, C * G], f32))
        dest_f = ec(nc.sbuf_tensor("dest_f", [P, C], f32))
        dest_i = ec(nc.sbuf_tensor("dest_i", [P, C], i32))
        destw_t = ec(nc.sbuf_tensor("destw_t", [P, N // 16], i16))
        dummy_idx = ec(nc.sbuf_tensor("dummy_idx", [P, 1], i16))
        dummy_pay = ec(nc.sbuf_tensor("dummy_pay", [P, 1], f32))
        xt = ec(nc.sbuf_tensor("xt", [P, C * D], f32))
        a_ps = ec(nc.psum_tensor("a_ps", [P, G], f32))
        ps_w = ec(nc.psum_tensor("ps_w", [P, C * 8], f32))
        s_const = ec(nc.semaphore("s_const"))
        s_oh = ec(nc.semaphore("s_oh"))
        s_xl_sp = ec(nc.semaphore("s_xl_sp"))
        s_xl_act = ec(nc.semaphore("s_xl_act"))
        s_scat = ec(nc.semaphore("s_scat"))
        s_sa = ec(nc.semaphore("s_sa"))
        s_warm = ec(nc.semaphore("s_warm"))
        s_dve = ec(nc.semaphore("s_dve"))
        s_pe = ec(nc.semaphore("s_pe"))
        su_t = cst_big_t[:, 0:P]
        repsel = [cst_big_t[:, (1 + t) * P:(2 + t) * P] for t in range(8)]
        ones_t = cst_row_t[:, 0:P]
        cst_t = cst_row_t[:, P:P + G]

        # sem target for "column c has landed", per ring segment
        col_sem = {}
        for i, (c0, c1) in enumerate(act_segs):
            for c in range(c0, c1):
                col_sem[c] = (s_xl_act, 16 * (i + 1))
        for i, (c0, c1) in enumerate(sp_segs):
            for c in range(c0, c1):
                col_sem[c] = (s_xl_sp, 16 * (i + 1))
        waited = {s_xl_act: 0, s_xl_sp: 0}

        def wait_cols(c0, c1):
            for c in range(c0, c1):
                sem, val = col_sem[c]
                if waited[sem] < val:
                    nc.gpsimd.wait_ge(sem, val)
                    waited[sem] = val

        # ---------------- plain DMAs ----------------
        nc.sync.dma_start(
            out=oh_t[:],
            in_=oh_d[:].rearrange("(p c) g -> p (c g)", p=P)).then_inc(
            s_oh, 16)
        nc.scalar.dma_start(out=cst_big_t[:], in_=cst_big_d[:]).then_inc(
            s_const, 16)
        nc.scalar.dma_start(out=cst_row_t[:], in_=cst_row_d[:]).then_inc(
            s_const, 16)
        # p-major: token n = p*64 + c lives at partition p, column c.
        x3 = x_d[:].rearrange("(p c) d -> p c d", p=P)
        xto = xt[:].rearrange("p (c d) -> p c d", d=D)
        for c0, c1 in act_segs:
            nc.scalar.dma_start(
                out=xto[:, c0:c1, :],
                in_=x3[:, c0:c1, :]).then_inc(s_xl_act, 16)
        for c0, c1 in sp_segs:
            nc.sync.dma_start(
                out=xto[:, c0:c1, :],
                in_=x3[:, c0:c1, :]).then_inc(s_xl_sp, 16)

        # ---------------- DVE: index pipeline ----------------
        nc.vector.wait_ge(s_oh, 16)
        for g in range(G):
            ins = nc.vector.tensor_tensor_scan(
                out=scan_t[:, g::G], data0=oh_t[:, g::G],
                data1=oh_t[:, g::G], initial=0.0,
                op0=mybir.AluOpType.add, op1=mybir.AluOpType.bypass)
            if g == G - 1:
                ins.then_inc(s_dve, 1)
        nc.vector.wait_ge(s_pe, 1)
        a_bcast = a_ps[:].unsqueeze(1).to_broadcast([P, C, G])
        nc.vector.tensor_tensor(
            out=s_t[:].rearrange("p (c g) -> p c g", g=G),
            in0=scan_t[:].rearrange("p (c g) -> p c g", g=G),
            in1=a_bcast, op=mybir.AluOpType.add)
        nc.vector.tensor_tensor(out=prod_t[:], in0=oh_t[:], in1=s_t[:],
                                op=mybir.AluOpType.mult)
        nc.vector.tensor_reduce(
            out=dest_f[:],
            in_=prod_t[:].rearrange("p (c g) -> p c g", g=G),
            axis=mybir.AxisListType.X,
            op=mybir.AluOpType.add)
        nc.vector.tensor_copy(out=dest_i[:], in_=dest_f[:]).then_inc(
            s_dve, 1)
        # int16 SWDGE index fold for the two scatter_add chunks
        nc.vector.wait_ge(s_pe, 2)
        nc.vector.tensor_copy(
            out=destw_t[:].rearrange("q (c t) -> q c t", t=8),
            in_=ps_w[:].rearrange("q (t c) -> q c t", c=C)).then_inc(
            s_dve, 1)

        # ---------------- PE ----------------
        nc.tensor.wait_ge(s_const, 32)
        nc.tensor.wait_ge(s_dve, 1)
        rowtot = scan_t[:, (C - 1) * G: C * G]
        nc.tensor.matmul(out=a_ps[:], lhsT=su_t, rhs=rowtot,
                         start=True, stop=False)
        nc.tensor.matmul(out=a_ps[:], lhsT=ones_t, rhs=cst_t,
                         start=False, stop=True).then_inc(s_pe, 1)
        nc.tensor.wait_ge(s_dve, 2)
        for t in range(8):
            ins = nc.tensor.matmul(out=ps_w[:, t * C:(t + 1) * C],
                                   lhsT=repsel[t],
                                   rhs=dest_f[:], start=True, stop=True)
        ins.then_inc(s_pe, 1)

        # ---------------- Pool: scatters ----------------
        nidx_reg = nc.gpsimd.to_reg(GCH)
        xt3 = xt[:].rearrange("p (c d) -> p c d", d=D)

        def indirect(c):
            wait_cols(c, c + 1)
            nc.gpsimd.indirect_dma_start(
                out=out_d[:],
                out_offset=bass.IndirectOffsetOnAxis(
                    ap=dest_i[:, c:c + 1], axis=0),
                in_=xt3[:, c, :],
                in_offset=None,
            ).then_inc(s_scat, 16)

        nc.gpsimd.wait_ge(s_dve, 2)  # dest_i ready
        # columns 16..23 first (their loads land first)
        for c in range(16, 24):
            indirect(c)
        # scatter_add chunk A: columns 0..7
        nc.gpsimd.wait_ge(s_dve, 3)  # destw ready
        wait_cols(0, 8)
        nc.gpsimd.dma_scatter_add(
            out_d[:], xt3[:, 0:8, :], destw_t[:, 0:64],
            GCH, nidx_reg, D, queue_num=1).then_inc(s_sa, 16)
        for c in range(28, 32):
            indirect(c)
        for c in range(24, 28):
            indirect(c)
        # scatter_add chunk B: columns 8..15
        wait_cols(8, 16)
        nc.gpsimd.dma_scatter_add(
            out_d[:], xt3[:, 8:16, :], destw_t[:, 64:128],
            GCH, nidx_reg, D, queue_num=2).then_inc(s_sa, 16)
        for c in range(32, C):
            indirect(c)
        nc.gpsimd.wait_ge(s_scat, 16 * 48)
        nc.gpsimd.wait_ge(s_sa, 32)

    nc.compile()
    return nc


def _get_nc():
    global _cached
    if _cached is None:
        _cached = _build()
    return _cached


def _constants():
    su = np.triu(np.ones((P, P), np.float32), k=1)
    rs = []
    for t in range(8):
        m = np.zeros((P, P), np.float32)
        for mm in range(8):
            for q in range(16):
                m[t * 16 + q, mm * 16 + q] = 1.0
        rs.append(m)
    cst_big = np.concatenate([su] + rs, axis=1)
    ones_r = np.ones((1, P), np.float32)
    cst = (np.arange(G, dtype=np.float32) * CAP - 1.0).reshape(1, G)
    cst_row = np.concatenate([ones_r, cst], axis=1)
    return cst_big, cst_row


def kernel(x, block_onehot, capacity):
    from concourse.bass_utils import run_bass_kernel_spmd

    x = np.ascontiguousarray(np.asarray(x, dtype=np.float32))
    oh = np.asarray(block_onehot, dtype=np.float32)
    if oh.ndim == 2:
        oh = np.broadcast_to(oh[None], (B,) + oh.shape)
    oh = np.ascontiguousarray(oh)
    assert x.shape == (B, N, D), x.shape
    assert oh.shape == (B, N, G), oh.shape
    assert int(capacity) == CAP, capacity
    nc = _get_nc()
    cst_big, cst_row = _constants()
    in_maps = [
        {"x": x[b], "oh": oh[b], "cst_big": cst_big, "cst_row": cst_row}
        for b in range(B)
    ]
    res = run_bass_kernel_spmd(nc, in_maps, core_ids=list(range(NCORES)))
    return np.stack([res.results[b]["out"].reshape(G, CAP, D)
                     for b in range(B)])


# revision 20
# speedup vs baseline: 1.0322x; 1.0322x over previous
"""BlockGrouper (MoE routing dispatch) Trainium2 kernel — raw bass.

Semantics (from the reference): each token n in sample b belongs to group
g = argmax(block_onehot[b, n]); its slot within the group is its rank
among same-group tokens in token order.  With the balanced one-hot
routing, the output [B, G, cap, D] is a pure row-permutation of
x [B, N, D].

Sharding: data-parallel over B across the 8 NeuronCores (one sample per
core); each core moves 16 MiB in + 16 MiB out.

Per-core program (N=8192, G=16, D=512, cap=512, P=128, C=64; token n
lives at partition p = n // 64, column c = n % 64):
  1. Index pipeline: dest[n] = g*cap + rank(n)
     = sum_g onehot * (prefix_c + carry_p + g*cap - 1), where prefix_c is
     a per-partition inclusive prefix sum along c (16 strided
     tensor_tensor_scan ops) and carry_p comes from one
     strict-upper-triangular-ones matmul on the per-partition totals
     (plus a const-row matmul adding g*cap - 1).  dest_f (f32) is cast
     to dest_i (int32) for the indirect scatters, and folded to the
     int16 SWDGE layout (destw) for the two dma_scatter_add chunks.
  2. Data path, two mechanisms that attack different bottlenecks:
     - 48 columns go out via indirect scatter-WRITE DMAs
       (indirect_dma_start, cce bypass): one plain 2 KiB descriptor per
       row, minimal DMA-engine work, but each 128-row call costs
       ~1.4 us of Pool-engine issue time (the critical path).
     - 16 columns (c=0..15) go out via two canned dma_scatter_add ops
       (1024 rows per instruction, ~2 us issue each) on SWDGE queues
       1/2 while the indirect stream keeps queue 0 — trading some CCE
       read-modify-write DMA work (engines have headroom) to shorten
       the serialized issue stream by ~23 us.
     x loads are 1 MiB groups (4 columns, 8 KiB contiguous per
     partition) ordered so the columns the scatter stream consumes
     first arrive first (g4,g6,g0,g2,... on ACT; g5,g7,g1,g3,... on SP
     behind the onehot load).
  A dummy 16-index scatter at t=0 preloads the Q7 extended-instruction
  library (LOAD_LIB costs ~13 us if taken lazily).
"""


import numpy as np

B, N, G, D = 8, 8192, 16, 512
CAP = N // G
P = 128
C = N // P
NCORES = 8
NG = 16          # load groups
GC = C // NG     # 4 columns per load group
GCH = 1024       # rows per dma_scatter_add chunk (8 columns)

_cached = None


def _build():
    import concourse.bass as bass
    import concourse.bacc as bacc
    import concourse.mybir as mybir

    f32 = mybir.dt.float32
    i32 = mybir.dt.int32
    i16 = mybir.dt.int16

    nc = bacc.Bacc("TRN2", target_bir_lowering=False, debug=False,
                   num_devices=NCORES, num_swdge_queues=4)
    x_d = nc.dram_tensor("x", [N, D], f32, kind="ExternalInput")
    oh_d = nc.dram_tensor("oh", [N, G], f32, kind="ExternalInput")
    cst_big_d = nc.dram_tensor("cst_big", [P, 9 * P], f32,
                               kind="ExternalInput")
    cst_row_d = nc.dram_tensor("cst_row", [1, P + G], f32,
                               kind="ExternalInput")
    out_d = nc.dram_tensor("out", [N, D], f32, kind="ExternalOutput")
    dummy_d = nc.dram_tensor("lib_warm", [16, 64], f32,
                             kind="ExternalOutput")

    # load segments (column ranges) per ring, in emission order: the
    # columns the scatter stream consumes first (16..23) land first in
    # small 4-column pieces; the dma_scatter_add chunks (0..15) next;
    # the rest as big 8-column segments.  Few DMA instructions keep the
    # program-init shape-register cost down.
    act_segs = [(16, 20), (0, 8), (24, 28), (32, 40), (48, 56)]
    sp_segs = [(20, 24), (8, 16), (28, 32), (40, 48), (56, 64)]

    from contextlib import ExitStack
    with ExitStack() as stack:
        ec = stack.enter_context
        cst_big_t = ec(nc.sbuf_tensor("cst_big_t", [P, 9 * P], f32))
        cst_row_t = ec(nc.sbuf_tensor("cst_row_t", [1, P + G], f32))
        oh_t = ec(nc.sbuf_tensor("oh_t", [P, C * G], f32))
        scan_t = ec(nc.sbuf_tensor("scan_t", [P, C * G], f32))
        s_t = ec(nc.sbuf_tensor("s_t", [P, C * G], f32))
        prod_t = ec(nc.sbuf_tensor("prod_t", [P, C * G], f32))
        dest_f = ec(nc.sbuf_tensor("dest_f", [P, C], f32))
        dest_i = ec(nc.sbuf_tensor("dest_i", [P, C], i32))
        destw_t = ec(nc.sbuf_tensor("destw_t", [P, N // 16], i16))
        dummy_idx = ec(nc.sbuf_tensor("dummy_idx", [P, 1], i16))
        dummy_pay = ec(nc.sbuf_tensor("dummy_pay", [P, 1], f32))
        xt = ec(nc.sbuf_tensor("xt", [P, C * D], f32))
        a_ps = ec(nc.psum_tensor("a_ps", [P, G], f32))
        ps_w = ec(nc.psum_tensor("ps_w", [P, C * 8], f32))
        s_const = ec(nc.semaphore("s_const"))
        s_oh = ec(nc.semaphore("s_oh"))
        s_xl_sp = ec(nc.semaphore("s_xl_sp"))
        s_xl_act = ec(nc.semaphore("s_xl_act"))
        s_scat = ec(nc.semaphore("s_scat"))
        s_sa = ec(nc.semaphore("s_sa"))
        s_warm = ec(nc.semaphore("s_warm"))
        s_dve = ec(nc.semaphore("s_dve"))
        s_pe = ec(nc.semaphore("s_pe"))
        su_t = cst_big_t[:, 0:P]
        repsel = [cst_big_t[:, (1 + t) * P:(2 + t) * P] for t in range(8)]
        ones_t = cst_row_t[:, 0:P]
        cst_t = cst_row_t[:, P:P + G]

        # sem target for "column c has landed", per ring segment
        col_sem = {}
        for i, (c0, c1) in enumerate(act_segs):
            for c in range(c0, c1):
                col_sem[c] = (s_xl_act, 16 * (i + 1))
        for i, (c0, c1) in enumerate(sp_segs):
            for c in range(c0, c1):
                col_sem[c] = (s_xl_sp, 16 * (i + 1))
        waited = {s_xl_act: 0, s_xl_sp: 0}

        def wait_cols(c0, c1):
            for c in range(c0, c1):
                sem, val = col_sem[c]
                if waited[sem] < val:
                    nc.gpsimd.wait_ge(sem, val)
                    waited[sem] = val

        # ---------------- plain DMAs ----------------
        nc.sync.dma_start(
            out=oh_t[:],
            in_=oh_d[:].rearrange("(p c) g -> p (c g)", p=P)).then_inc(
            s_oh, 16)
        nc.scalar.dma_start(out=cst_big_t[:], in_=cst_big_d[:]).then_inc(
            s_const, 16)
        nc.scalar.dma_start(out=cst_row_t[:], in_=cst_row_d[:]).then_inc(
            s_const, 16)
        # p-major: token n = p*64 + c lives at partition p, column c.
        x3 = x_d[:].rearrange("(p c) d -> p c d", p=P)
        xto = xt[:].rearrange("p (c d) -> p c d", d=D)
        for c0, c1 in act_segs:
            nc.scalar.dma_start(
                out=xto[:, c0:c1, :],
                in_=x3[:, c0:c1, :]).then_inc(s_xl_act, 16)
        for c0, c1 in sp_segs:
            nc.sync.dma_start(
                out=xto[:, c0:c1, :],
                in_=x3[:, c0:c1, :]).then_inc(s_xl_sp, 16)

        # ---------------- DVE: index pipeline ----------------
        nc.vector.wait_ge(s_oh, 16)
        for g in range(G):
            ins = nc.vector.tensor_tensor_scan(
                out=scan_t[:, g::G], data0=oh_t[:, g::G],
                data1=oh_t[:, g::G], initial=0.0,
                op0=mybir.AluOpType.add, op1=mybir.AluOpType.bypass)
            if g == G - 1:
                ins.then_inc(s_dve, 1)
        nc.vector.wait_ge(s_pe, 1)
        a_bcast = a_ps[:].unsqueeze(1).to_broadcast([P, C, G])
        nc.vector.tensor_tensor(
            out=s_t[:].rearrange("p (c g) -> p c g", g=G),
            in0=scan_t[:].rearrange("p (c g) -> p c g", g=G),
            in1=a_bcast, op=mybir.AluOpType.add)
        nc.vector.tensor_tensor(out=prod_t[:], in0=oh_t[:], in1=s_t[:],
                                op=mybir.AluOpType.mult)
        nc.vector.tensor_reduce(
            out=dest_f[:],
            in_=prod_t[:].rearrange("p (c g) -> p c g", g=G),
            axis=mybir.AxisListType.X,
            op=mybir.AluOpType.add)
        nc.vector.tensor_copy(out=dest_i[:], in_=dest_f[:]).then_inc(
            s_dve, 1)
        # int16 SWDGE index fold for the two scatter_add chunks
        nc.vector.wait_ge(s_pe, 2)
        nc.vector.tensor_copy(
            out=destw_t[:].rearrange("q (c t) -> q c t", t=8),
            in_=ps_w[:].rearrange("q (t c) -> q c t", c=C)).then_inc(
            s_dve, 1)

        # ---------------- PE ----------------
        nc.tensor.wait_ge(s_const, 32)
        nc.tensor.wait_ge(s_dve, 1)
        rowtot = scan_t[:, (C - 1) * G: C * G]
        nc.tensor.matmul(out=a_ps[:], lhsT=su_t, rhs=rowtot,
                         start=True, stop=False)
        nc.tensor.matmul(out=a_ps[:], lhsT=ones_t, rhs=cst_t,
                         start=False, stop=True).then_inc(s_pe, 1)
        nc.tensor.wait_ge(s_dve, 2)
        for t in range(8):
            ins = nc.tensor.matmul(out=ps_w[:, t * C:(t + 1) * C],
                                   lhsT=repsel[t],
                                   rhs=dest_f[:], start=True, stop=True)
        ins.then_inc(s_pe, 1)

        # ---------------- Pool: scatters ----------------
        # one dummy 16-idx scatter at t=0 pulls the Q7 extended-inst
        # LOAD_LIB (global) during the otherwise-idle load phase
        nc.gpsimd.memset(dummy_idx[:], 0)
        nc.gpsimd.dma_scatter_add(
            dummy_d[:][:, 0:1],
            dummy_pay[:].rearrange("p (c one) -> p c one", one=1),
            dummy_idx[:], 16, 16, 1, elem_step=64,
            queue_num=3).then_inc(s_warm, 16)
        nidx_reg = nc.gpsimd.to_reg(GCH)
        xt3 = xt[:].rearrange("p (c d) -> p c d", d=D)

        def indirect(c):
            wait_cols(c, c + 1)
            nc.gpsimd.indirect_dma_start(
                out=out_d[:],
                out_offset=bass.IndirectOffsetOnAxis(
                    ap=dest_i[:, c:c + 1], axis=0),
                in_=xt3[:, c, :],
                in_offset=None,
            ).then_inc(s_scat, 16)

        nc.gpsimd.wait_ge(s_dve, 2)  # dest_i ready
        # columns 16..23 first (their loads land first)
        for c in range(16, 24):
            indirect(c)
        # scatter_add chunk A: columns 0..7
        nc.gpsimd.wait_ge(s_dve, 3)  # destw ready
        wait_cols(0, 8)
        nc.gpsimd.dma_scatter_add(
            out_d[:], xt3[:, 0:8, :], destw_t[:, 0:64],
            GCH, nidx_reg, D, queue_num=1).then_inc(s_sa, 16)
        for c in range(28, 32):
            indirect(c)
        for c in range(24, 28):
            indirect(c)
        # scatter_add chunk B: columns 8..15
        wait_cols(8, 16)
        nc.gpsimd.dma_scatter_add(
            out_d[:], xt3[:, 8:16, :], destw_t[:, 64:128],
            GCH, nidx_reg, D, queue_num=2).then_inc(s_sa, 16)
        # scatter_add chunk C: columns 32..39 — issued right after B so
        # its (slow, ~25 rows/us) per-queue drain finishes well before
        # the indirect issue stream ends
        wait_cols(32, 40)
        nc.gpsimd.dma_scatter_add(
            out_d[:], xt3[:, 32:40, :], destw_t[:, 256:320],
            GCH, nidx_reg, D, queue_num=3).then_inc(s_sa, 16)
        for c in range(40, C):
            indirect(c)
        nc.gpsimd.wait_ge(s_scat, 16 * 40)
        nc.gpsimd.wait_ge(s_sa, 48)
        nc.gpsimd.wait_ge(s_warm, 16)

    nc.compile()
    return nc


def _get_nc():
    global _cached
    if _cached is None:
        _cached = _build()
    return _cached


def _constants():
    su = np.triu(np.ones((P, P), np.float32), k=1)
    rs = []
    for t in range(8):
        m = np.zeros((P, P), np.float32)
        for mm in range(8):
            for q in range(16):
                m[t * 16 + q, mm * 16 + q] = 1.0
        rs.append(m)
    cst_big = np.concatenate([su] + rs, axis=1)
    ones_r = np.ones((1, P), np.float32)
    cst = (np.arange(G, dtype=np.float32) * CAP - 1.0).reshape(1, G)
    cst_row = np.concatenate([ones_r, cst], axis=1)
    return cst_big, cst_row


def kernel(x, block_onehot, capacity):
    from concourse.bass_utils import run_bass_kernel_spmd

    x = np.ascontiguousarray(np.asarray(x, dtype=np.float32))
    oh = np.asarray(block_onehot, dtype=np.float32)
    if oh.ndim == 2:
        oh = np.broadcast_to(oh[None], (B,) + oh.shape)
    oh = np.ascontiguousarray(oh)
    assert x.shape == (B, N, D), x.shape
    assert oh.shape == (B, N, G), oh.shape
    assert int(capacity) == CAP, capacity
    nc = _get_nc()
    cst_big, cst_row = _constants()
    in_maps = [
        {"x": x[b], "oh": oh[b], "cst_big": cst_big, "cst_row": cst_row}
        for b in range(B)
    ]
    res = run_bass_kernel_spmd(nc, in_maps, core_ids=list(range(NCORES)))
    return np.stack([res.results[b]["out"].reshape(G, CAP, D)
                     for b in range(B)])


# revision 22
# speedup vs baseline: 1.1264x; 1.0913x over previous
"""BlockGrouper (MoE routing dispatch) Trainium2 kernel — raw bass.

Semantics (from the reference): each token n in sample b belongs to group
g = argmax(block_onehot[b, n]); its slot within the group is its rank
among same-group tokens in token order.  With the balanced one-hot
routing, the output [B, G, cap, D] is a pure row-permutation of
x [B, N, D].

Sharding: data-parallel over B across the 8 NeuronCores (one sample per
core); each core moves 16 MiB in + 16 MiB out.

Per-core program (N=8192, G=16, D=512, cap=512, P=128, C=64; token n
lives at partition p = n // 64, column c = n % 64):
  1. Index pipeline: dest[n] = g*cap + rank(n)
     = sum_g onehot * (prefix_c + carry_p + g*cap - 1), where prefix_c is
     a per-partition inclusive prefix sum along c (16 strided
     tensor_tensor_scan ops) and carry_p comes from one
     strict-upper-triangular-ones matmul on the per-partition totals
     (plus a const-row matmul adding g*cap - 1).  dest_f (f32) is cast
     to dest_i (int32) for the indirect scatters, and folded to the
     int16 SWDGE layout (destw) for the two dma_scatter_add chunks.
  2. Data path, two mechanisms that attack different bottlenecks:
     - 48 columns go out via indirect scatter-WRITE DMAs
       (indirect_dma_start, cce bypass): one plain 2 KiB descriptor per
       row, minimal DMA-engine work, but each 128-row call costs
       ~1.4 us of Pool-engine issue time (the critical path).
     - 16 columns (c=0..15) go out via two canned dma_scatter_add ops
       (1024 rows per instruction, ~2 us issue each) on SWDGE queues
       1/2 while the indirect stream keeps queue 0 — trading some CCE
       read-modify-write DMA work (engines have headroom) to shorten
       the serialized issue stream by ~23 us.
     x loads are 1 MiB groups (4 columns, 8 KiB contiguous per
     partition) ordered so the columns the scatter stream consumes
     first arrive first (g4,g6,g0,g2,... on ACT; g5,g7,g1,g3,... on SP
     behind the onehot load).
  A dummy 16-index scatter at t=0 preloads the Q7 extended-instruction
  library (LOAD_LIB costs ~13 us if taken lazily).
"""


import numpy as np

B, N, G, D = 8, 8192, 16, 512
CAP = N // G
P = 128
C = N // P
NCORES = 8
NG = 16          # load groups
GC = C // NG     # 4 columns per load group
GCH = 1024       # rows per dma_scatter_add chunk (8 columns)

_cached = None


def _build():
    import concourse.bass as bass
    import concourse.bacc as bacc
    import concourse.mybir as mybir

    f32 = mybir.dt.float32
    i32 = mybir.dt.int32
    i16 = mybir.dt.int16

    nc = bacc.Bacc("TRN2", target_bir_lowering=False, debug=False,
                   num_devices=NCORES, num_swdge_queues=4)
    x_d = nc.dram_tensor("x", [N, D], f32, kind="ExternalInput")
    oh_d = nc.dram_tensor("oh", [N, G], f32, kind="ExternalInput")
    cst_big_d = nc.dram_tensor("cst_big", [P, 9 * P], f32,
                               kind="ExternalInput")
    cst_row_d = nc.dram_tensor("cst_row", [1, P + G], f32,
                               kind="ExternalInput")
    out_d = nc.dram_tensor("out", [N, D], f32, kind="ExternalOutput")
    dummy_d = nc.dram_tensor("lib_warm", [16, 64], f32,
                             kind="ExternalOutput")

    # load segments (column ranges) per ring, in emission order: the
    # columns the scatter stream consumes first (16..23) land first in
    # small 4-column pieces; the dma_scatter_add chunks (0..15) next;
    # the rest as big 8-column segments.  Few DMA instructions keep the
    # program-init shape-register cost down.
    act_segs = [(16, 20), (0, 8), (24, 28), (32, 40), (48, 56)]
    sp_segs = [(20, 24), (8, 16), (28, 32), (40, 48), (56, 64)]

    from contextlib import ExitStack
    with ExitStack() as stack:
        ec = stack.enter_context
        cst_big_t = ec(nc.sbuf_tensor("cst_big_t", [P, 9 * P], f32))
        cst_row_t = ec(nc.sbuf_tensor("cst_row_t", [1, P + G], f32))
        oh_t = ec(nc.sbuf_tensor("oh_t", [P, C * G], f32))
        scan_t = ec(nc.sbuf_tensor("scan_t", [P, C * G], f32))
        s_t = ec(nc.sbuf_tensor("s_t", [P, C * G], f32))
        prod_t = ec(nc.sbuf_tensor("prod_t", [P, C * G], f32))
        dest_f = ec(nc.sbuf_tensor("dest_f", [P, C], f32))
        dest_i = ec(nc.sbuf_tensor("dest_i", [P, C], i32))
        destw_t = ec(nc.sbuf_tensor("destw_t", [P, N // 16], i16))
        dummy_idx = ec(nc.sbuf_tensor("dummy_idx", [P, 1], i16))
        dummy_pay = ec(nc.sbuf_tensor("dummy_pay", [P, 1], f32))
        xt = ec(nc.sbuf_tensor("xt", [P, C * D], f32))
        a_ps = ec(nc.psum_tensor("a_ps", [P, G], f32))
        ps_w = ec(nc.psum_tensor("ps_w", [P, C * 8], f32))
        s_const = ec(nc.semaphore("s_const"))
        s_oh = ec(nc.semaphore("s_oh"))
        s_xl_sp = ec(nc.semaphore("s_xl_sp"))
        s_xl_act = ec(nc.semaphore("s_xl_act"))
        s_scat = ec(nc.semaphore("s_scat"))
        s_sa = ec(nc.semaphore("s_sa"))
        s_warm = ec(nc.semaphore("s_warm"))
        s_dve = ec(nc.semaphore("s_dve"))
        s_pe = ec(nc.semaphore("s_pe"))
        su_t = cst_big_t[:, 0:P]
        repsel = [cst_big_t[:, (1 + t) * P:(2 + t) * P] for t in range(8)]
        ones_t = cst_row_t[:, 0:P]
        cst_t = cst_row_t[:, P:P + G]

        # sem target for "column c has landed", per ring segment
        col_sem = {}
        for i, (c0, c1) in enumerate(act_segs):
            for c in range(c0, c1):
                col_sem[c] = (s_xl_act, 16 * (i + 1))
        for i, (c0, c1) in enumerate(sp_segs):
            for c in range(c0, c1):
                col_sem[c] = (s_xl_sp, 16 * (i + 1))
        waited = {s_xl_act: 0, s_xl_sp: 0}

        def wait_cols(c0, c1):
            for c in range(c0, c1):
                sem, val = col_sem[c]
                if waited[sem] < val:
                    nc.gpsimd.wait_ge(sem, val)
                    waited[sem] = val

        # ---------------- plain DMAs ----------------
        nc.sync.dma_start(
            out=oh_t[:],
            in_=oh_d[:].rearrange("(p c) g -> p (c g)", p=P)).then_inc(
            s_oh, 16)
        nc.scalar.dma_start(out=cst_big_t[:], in_=cst_big_d[:]).then_inc(
            s_const, 16)
        nc.scalar.dma_start(out=cst_row_t[:], in_=cst_row_d[:]).then_inc(
            s_const, 16)
        # p-major: token n = p*64 + c lives at partition p, column c.
        x3 = x_d[:].rearrange("(p c) d -> p c d", p=P)
        xto = xt[:].rearrange("p (c d) -> p c d", d=D)
        for c0, c1 in act_segs:
            nc.scalar.dma_start(
                out=xto[:, c0:c1, :],
                in_=x3[:, c0:c1, :]).then_inc(s_xl_act, 16)
        for c0, c1 in sp_segs:
            nc.sync.dma_start(
                out=xto[:, c0:c1, :],
                in_=x3[:, c0:c1, :]).then_inc(s_xl_sp, 16)

        # ---------------- DVE: index pipeline ----------------
        nc.vector.wait_ge(s_oh, 16)
        for g in range(G):
            ins = nc.vector.tensor_tensor_scan(
                out=scan_t[:, g::G], data0=oh_t[:, g::G],
                data1=oh_t[:, g::G], initial=0.0,
                op0=mybir.AluOpType.add, op1=mybir.AluOpType.bypass)
            if g == G - 1:
                ins.then_inc(s_dve, 1)
        nc.vector.wait_ge(s_pe, 1)
        a_bcast = a_ps[:].unsqueeze(1).to_broadcast([P, C, G])
        nc.vector.tensor_tensor(
            out=s_t[:].rearrange("p (c g) -> p c g", g=G),
            in0=scan_t[:].rearrange("p (c g) -> p c g", g=G),
            in1=a_bcast, op=mybir.AluOpType.add)
        nc.vector.tensor_tensor(out=prod_t[:], in0=oh_t[:], in1=s_t[:],
                                op=mybir.AluOpType.mult)
        nc.vector.tensor_reduce(
            out=dest_f[:],
            in_=prod_t[:].rearrange("p (c g) -> p c g", g=G),
            axis=mybir.AxisListType.X,
            op=mybir.AluOpType.add)
        nc.vector.tensor_copy(out=dest_i[:], in_=dest_f[:]).then_inc(
            s_dve, 1)
        # int16 SWDGE index fold for the two scatter_add chunks
        nc.vector.wait_ge(s_pe, 2)
        nc.vector.tensor_copy(
            out=destw_t[:].rearrange("q (c t) -> q c t", t=8),
            in_=ps_w[:].rearrange("q (t c) -> q c t", c=C)).then_inc(
            s_dve, 1)

        # ---------------- PE ----------------
        nc.tensor.wait_ge(s_const, 32)
        nc.tensor.wait_ge(s_dve, 1)
        rowtot = scan_t[:, (C - 1) * G: C * G]
        nc.tensor.matmul(out=a_ps[:], lhsT=su_t, rhs=rowtot,
                         start=True, stop=False)
        nc.tensor.matmul(out=a_ps[:], lhsT=ones_t, rhs=cst_t,
                         start=False, stop=True).then_inc(s_pe, 1)
        nc.tensor.wait_ge(s_dve, 2)
        for t in range(8):
            ins = nc.tensor.matmul(out=ps_w[:, t * C:(t + 1) * C],
                                   lhsT=repsel[t],
                                   rhs=dest_f[:], start=True, stop=True)
        ins.then_inc(s_pe, 1)

        # ---------------- Pool: scatters ----------------
        # one dummy 16-idx scatter at t=0 pulls the Q7 extended-inst
        # LOAD_LIB (global across queues) during the otherwise-idle
        # load phase; without it the first real dma_scatter_add pays
        # ~13 us inline and blocks the whole issue stream
        nc.gpsimd.memset(dummy_idx[:], 0)
        nc.gpsimd.dma_scatter_add(
            dummy_d[:][:, 0:1],
            dummy_pay[:].rearrange("p (c one) -> p c one", one=1),
            dummy_idx[:], 16, 16, 1, elem_step=64,
            queue_num=1).then_inc(s_warm, 16)
        nidx_reg = nc.gpsimd.to_reg(GCH)
        xt3 = xt[:].rearrange("p (c d) -> p c d", d=D)

        def indirect(c):
            wait_cols(c, c + 1)
            nc.gpsimd.indirect_dma_start(
                out=out_d[:],
                out_offset=bass.IndirectOffsetOnAxis(
                    ap=dest_i[:, c:c + 1], axis=0),
                in_=xt3[:, c, :],
                in_offset=None,
            ).then_inc(s_scat, 16)

        nc.gpsimd.wait_ge(s_dve, 2)  # dest_i ready
        # columns 16..23 first (their loads land first)
        for c in range(16, 24):
            indirect(c)
        # scatter_add chunk A: columns 0..7
        nc.gpsimd.wait_ge(s_dve, 3)  # destw ready
        wait_cols(0, 8)
        nc.gpsimd.dma_scatter_add(
            out_d[:], xt3[:, 0:8, :], destw_t[:, 0:64],
            GCH, nidx_reg, D, queue_num=1).then_inc(s_sa, 16)
        for c in range(28, 32):
            indirect(c)
        for c in range(24, 28):
            indirect(c)
        # scatter_add chunk B: columns 8..15
        wait_cols(8, 16)
        nc.gpsimd.dma_scatter_add(
            out_d[:], xt3[:, 8:16, :], destw_t[:, 64:128],
            GCH, nidx_reg, D, queue_num=2).then_inc(s_sa, 16)
        for c in range(32, C):
            indirect(c)
        nc.gpsimd.wait_ge(s_scat, 16 * 48)
        nc.gpsimd.wait_ge(s_sa, 32)
        nc.gpsimd.wait_ge(s_warm, 16)

    nc.compile()
    return nc


def _get_nc():
    global _cached
    if _cached is None:
        _cached = _build()
    return _cached


def _constants():
    su = np.triu(np.ones((P, P), np.float32), k=1)
    rs = []
    for t in range(8):
        m = np.zeros((P, P), np.float32)
        for mm in range(8):
            for q in range(16):
                m[t * 16 + q, mm * 16 + q] = 1.0
        rs.append(m)
    cst_big = np.concatenate([su] + rs, axis=1)
    ones_r = np.ones((1, P), np.float32)
    cst = (np.arange(G, dtype=np.float32) * CAP - 1.0).reshape(1, G)
    cst_row = np.concatenate([ones_r, cst], axis=1)
    return cst_big, cst_row


def kernel(x, block_onehot, capacity):
    from concourse.bass_utils import run_bass_kernel_spmd

    x = np.ascontiguousarray(np.asarray(x, dtype=np.float32))
    oh = np.asarray(block_onehot, dtype=np.float32)
    if oh.ndim == 2:
        oh = np.broadcast_to(oh[None], (B,) + oh.shape)
    oh = np.ascontiguousarray(oh)
    assert x.shape == (B, N, D), x.shape
    assert oh.shape == (B, N, G), oh.shape
    assert int(capacity) == CAP, capacity
    nc = _get_nc()
    cst_big, cst_row = _constants()
    in_maps = [
        {"x": x[b], "oh": oh[b], "cst_big": cst_big, "cst_row": cst_row}
        for b in range(B)
    ]
    res = run_bass_kernel_spmd(nc, in_maps, core_ids=list(range(NCORES)))
    return np.stack([res.results[b]["out"].reshape(G, CAP, D)
                     for b in range(B)])
